# revision 3
# baseline (speedup 1.0000x reference)
"""Bahdanau attention Trainium2 kernel.

Problem (per full input):
    query [32, 1024], values [32, 2048, 1024], W1/W2 [1024, 1024],
    b1/b2 [1024], V [1024, 1], bv [1]
    kv    = values @ W1 + b1                       [B, S, U]
    q     = query @ W2 + b2                        [B, 1, U]
    score = tanh(kv + q) @ V + bv                  [B, S, 1]
    attn  = softmax(score, axis=1)                 [B, S, 1]
    ctx   = sum(attn * values, axis=1)             [B, D]
    returns (ctx, attn)

Sharding: data-parallel over batch, 4 batches per core on 8 cores.

Per-core device algorithm (all loops statically unrolled, Tile-scheduled):
  prologue:
    biasq[u, b] = (query_b @ W2 + b2 + b1)^T  via PE (f32r), tiny
    W1 cast to bf16 chunk layout [128d, 8dc, 1024u]
  per batch b:
    A: DMA values s-chunk [128s, 1024d] f32 -> cast bf16 -> 8x DMA-XBAR
       transpose into valuesT [128d, 8dc, 2048s] bf16
    B: kv^T[u, s] psum tiles [128u, 512s] = sum_dc W1c^T @ valuesT (bf16)
       tanhT = tanh(kv^T + biasq[:, uc, b]) via ACT, bf16
    C: score[1, s] = sum_uc V_c^T @ tanhT (bf16 matvec);
       e = exp(score) + row-sum via ACT accum_out; inv = 1/sum (DVE);
       attn_row = e * inv -> DMA out
       e^T chunks via ones-matmul trick -> eT_sb [128, 16]
    D: ctx[1, d] = sum_sc eT_sb[:,sc]^T @ values-chunk (f32r matmuls over
       re-fetched fp32 values), scaled by inv -> DMA out

bv is provably irrelevant (softmax shift invariance), so it is ignored.
Softmax max-subtraction skipped: |score| <= ||V||_1 ~ 26, exp is safe in f32.
"""

import numpy as np

import concourse.bacc as bacc
import concourse.tile as tile
from concourse import mybir
from concourse.bass_utils import run_bass_kernel_spmd
from concourse.masks import make_identity

F32 = mybir.dt.float32
F32R = mybir.dt.float32r
BF16 = mybir.dt.bfloat16
F16 = mybir.dt.float16
AF = mybir.ActivationFunctionType

B, S, D, U = 32, 2048, 1024, 1024
NCORES = 8
BPC = B // NCORES           # batches per core
NSC = S // 128              # 16 s-chunks of 128
NSB = S // 512              # 4 s-blocks of 512
NDC = D // 128              # 8 d-chunks
NUC = U // 128              # 8 u-chunks


def build_nc():
    nc = bacc.Bacc(debug=False)

    values_h = nc.dram_tensor("values", [BPC, S, D], F32, kind="ExternalInput")
    query_h = nc.dram_tensor("query", [BPC, D], F32, kind="ExternalInput")
    w1_h = nc.dram_tensor("W1", [D, U], F32, kind="ExternalInput")
    w2_h = nc.dram_tensor("W2", [D, U], F32, kind="ExternalInput")
    b1_h = nc.dram_tensor("b1", [U], F32, kind="ExternalInput")
    b2_h = nc.dram_tensor("b2", [U], F32, kind="ExternalInput")
    v_h = nc.dram_tensor("V", [U, 1], F32, kind="ExternalInput")
    ctx_h = nc.dram_tensor("ctx_out", [BPC, D], F32, kind="ExternalOutput")
    attn_h = nc.dram_tensor("attn_out", [BPC, S], F32, kind="ExternalOutput")

    with tile.TileContext(nc) as tc:
        with (
            tc.tile_pool(name="consts", bufs=1) as consts,
            tc.tile_pool(name="vt", bufs=2) as vt_pool,
            tc.tile_pool(name="tanh", bufs=1) as tanh_pool,
            tc.tile_pool(name="stg", bufs=4) as stg_pool,
            tc.tile_pool(name="stgbf", bufs=4) as stgbf_pool,
            tc.tile_pool(name="stg2", bufs=4) as stg2_pool,
            tc.tile_pool(name="p2", bufs=4) as p2_pool,
            tc.tile_pool(name="rows", bufs=2) as rows,
            tc.tile_pool(name="pkv", bufs=3, space="PSUM") as psum_kv,
            tc.tile_pool(name="psmall", bufs=4, space="PSUM") as psum_small,
        ):
            # ---------------- prologue ----------------
            ident = consts.tile([BPC, BPC], F32)
            make_identity(nc, ident)
            ones = consts.tile([1, 1], F32)
            nc.vector.memset(ones, 1.0)

            # b1 + b2, transposed to [128u, 8uc] via strided DMA
            bt1 = consts.tile([128, NUC], F32)
            bt2 = consts.tile([128, NUC], F32)
            nc.scalar.dma_start(bt1, b1_h[:].rearrange("(c p) -> p c", p=128))
            nc.scalar.dma_start(bt2, b2_h[:].rearrange("(c p) -> p c", p=128))
            bt = consts.tile([128, NUC], F32)
            nc.vector.tensor_add(bt, bt1, bt2)

            # V transposed chunks [128u, 8uc], bf16
            v_f32 = consts.tile([128, NUC], F32)
            nc.scalar.dma_start(v_f32, v_h[:, :].rearrange("(c p) x -> p (c x)", p=128))
            v_mm = consts.tile([128, NUC], F16)
            nc.vector.tensor_copy(v_mm, v_f32)

            # query^T [128d, 8dc, BPC]
            q_sb = consts.tile([BPC, D], F32)
            nc.scalar.dma_start(q_sb, query_h[:, :])
            qt_sb = consts.tile([128, NDC, BPC], F32)
            for dc in range(NDC):
                pqt = psum_small.tile([128, BPC], F32, tag="small")
                nc.tensor.transpose(pqt, q_sb[:, dc * 128:(dc + 1) * 128],
                                    ident)
                nc.vector.tensor_copy(qt_sb[:, dc, :], pqt)

            # q @ W2 (transposed): qacc[128u, 8uc, BPC] = sum_dc W2c^T @ qT.
            # One single-shot matmul per (dc, uc) with SBUF-side accumulation:
            # interleaved PSUM accumulation groups sharing a bank are illegal
            # (start=True re-arms zeroing for the whole zero region).
            qacc = consts.tile([128, NUC, BPC], F32)
            for dc in range(NDC):
                w2t = stg_pool.tile([128, U], F32, tag="stg")
                nc.scalar.dma_start(w2t, w2_h[dc * 128:(dc + 1) * 128, :])
                for uc in range(NUC):
                    pp = psum_small.tile([128, BPC], F32, tag="small")
                    nc.tensor.matmul(
                        pp,
                        lhsT=w2t[:, uc * 128:(uc + 1) * 128],
                        rhs=qt_sb[:, dc, :],
                        start=True, stop=True,
                    )
                    if dc == 0:
                        nc.vector.tensor_copy(qacc[:, uc, :], pp)
                    else:
                        nc.vector.tensor_add(qacc[:, uc, :], qacc[:, uc, :], pp)
            # biasq = qacc + (b1+b2)^T broadcast over batch columns
            biasq = consts.tile([128, NUC, BPC], F32)
            for uc in range(NUC):
                nc.vector.tensor_scalar_add(biasq[:, uc, :], qacc[:, uc, :],
                                            bt[:, uc:uc + 1])

            # W1 -> bf16 chunk layout [128d, 8dc, 1024u]
            w1_mm = consts.tile([128, NDC, U], F16)
            for dc in range(NDC):
                w1t = stg_pool.tile([128, U], F32, tag="stg")
                nc.scalar.dma_start(w1t, w1_h[dc * 128:(dc + 1) * 128, :])
                nc.vector.tensor_copy(w1_mm[:, dc, :], w1t)

            # ---------------- main loop ----------------
            for b in range(BPC):
                # --- A: load + cast + transpose ---
                vt = vt_pool.tile([128, NDC, S], F16)
                for sc in range(NSC):
                    stg = stg_pool.tile([128, D], F32, tag="stg")
                    nc.scalar.dma_start(stg, values_h[b, sc * 128:(sc + 1) * 128, :])
                    sbf = stgbf_pool.tile([128, D], F16)
                    if sc % 2 == 0:
                        nc.vector.tensor_copy(sbf, stg)
                    else:
                        nc.scalar.copy(sbf, stg)
                    for dc in range(NDC):
                        nc.sync.dma_start_transpose(
                            vt[:, dc, sc * 128:(sc + 1) * 128],
                            sbf[:, dc * 128:(dc + 1) * 128])

                # --- B: kv^T + tanh ---
                tanh_t = tanh_pool.tile([128, NUC, S], F16)
                for uc in range(NUC):
                    for sb in range(NSB):
                        kv = psum_kv.tile([128, 512], F32, tag="kv")
                        for dc in range(NDC):
                            nc.tensor.matmul(
                                kv,
                                lhsT=w1_mm[:, dc, uc * 128:(uc + 1) * 128],
                                rhs=vt[:, dc, sb * 512:(sb + 1) * 512],
                                start=(dc == 0), stop=(dc == NDC - 1),
                            )
                        nc.scalar.activation(
                            tanh_t[:, uc, sb * 512:(sb + 1) * 512], kv,
                            AF.Tanh, bias=biasq[:, uc, b:b + 1], scale=1.0)

                # --- C: scores + softmax ---
                e_row = rows.tile([1, S], F32)
                esum_parts = rows.tile([1, NSB], F32)
                for sb in range(NSB):
                    scp = psum_small.tile([1, 512], F32, tag="small")
                    for uc in range(NUC):
                        nc.tensor.matmul(
                            scp,
                            lhsT=v_mm[:, uc:uc + 1],
                            rhs=tanh_t[:, uc, sb * 512:(sb + 1) * 512],
                            start=(uc == 0), stop=(uc == NUC - 1),
                        )
                    nc.scalar.activation(
                        e_row[0:1, sb * 512:(sb + 1) * 512], scp, AF.Exp,
                        accum_out=esum_parts[0:1, sb:sb + 1])
                tot = rows.tile([1, 1], F32)
                nc.vector.reduce_sum(out=tot, in_=esum_parts,
                                     axis=mybir.AxisListType.X)
                inv = rows.tile([1, 1], F32)
                nc.vector.reciprocal(inv, tot)
                w_row = rows.tile([1, S], F32)
                nc.scalar.mul(w_row, e_row, inv[0:1, 0:1])
                nc.scalar.dma_start(attn_h[b:b + 1, :], w_row)

                # e^T chunks [128, 16sc] via ones-matmul
                pet = psum_small.tile([128, NSC], F32, tag="small")
                for sc in range(NSC):
                    nc.tensor.matmul(
                        pet[:, sc:sc + 1],
                        lhsT=e_row[0:1, sc * 128:(sc + 1) * 128],
                        rhs=ones[0:1, 0:1],
                        start=True, stop=True, skip_group_check=True,
                    )
                et_sb = rows.tile([128, NSC], F16)
                nc.vector.tensor_copy(et_sb, pet)

                # --- D: context ---
                ctx0 = psum_small.tile([1, 512], F32, tag="small")
                ctx1 = psum_small.tile([1, 512], F32, tag="small")
                for sc in range(NSC):
                    stg2 = stg2_pool.tile([128, D], F32)
                    nc.scalar.dma_start(stg2, values_h[b, sc * 128:(sc + 1) * 128, :])
                    p2 = p2_pool.tile([128, D], F16)
                    if sc % 2 == 0:
                        nc.vector.tensor_copy(p2, stg2)
                    else:
                        nc.scalar.copy(p2, stg2)
                    lhs = et_sb[:, sc:sc + 1]
                    nc.tensor.matmul(ctx0, lhsT=lhs, rhs=p2[:, 0:512],
                                     start=(sc == 0), stop=(sc == NSC - 1),
                                     skip_group_check=True)
                    nc.tensor.matmul(ctx1, lhsT=lhs, rhs=p2[:, 512:D],
                                     start=(sc == 0), stop=(sc == NSC - 1),
                                     skip_group_check=True)
                ctx_row = rows.tile([1, D], F32)
                nc.scalar.mul(ctx_row[0:1, 0:512], ctx0, inv[0:1, 0:1])
                nc.scalar.mul(ctx_row[0:1, 512:D], ctx1, inv[0:1, 0:1])
                nc.scalar.dma_start(ctx_h[b:b + 1, :], ctx_row)

    nc.compile()
    return nc


_NC_CACHE = None


def _get_nc():
    global _NC_CACHE
    if _NC_CACHE is None:
        _NC_CACHE = build_nc()
    return _NC_CACHE


def make_in_maps(query, values, W1, b1, W2, b2, V, bv=None):
    del bv  # softmax shift-invariance: bv cannot affect either output
    asc = np.ascontiguousarray
    in_maps = []
    for c in range(NCORES):
        lo, hi = c * BPC, (c + 1) * BPC
        in_maps.append({
            "values": asc(values[lo:hi], dtype=np.float32),
            "query": asc(query[lo:hi], dtype=np.float32),
            "W1": asc(W1, dtype=np.float32),
            "W2": asc(W2, dtype=np.float32),
            "b1": asc(b1, dtype=np.float32),
            "b2": asc(b2, dtype=np.float32),
            "V": asc(V, dtype=np.float32),
        })
    return in_maps


def gather_outputs(results):
    ctx = np.concatenate([r["ctx_out"] for r in results], axis=0)
    attn = np.concatenate([r["attn_out"] for r in results], axis=0)
    return ctx.astype(np.float32), attn.reshape(B, S, 1).astype(np.float32)


def kernel(query, values, W1, b1, W2, b2, V, bv):
    nc = _get_nc()
    in_maps = make_in_maps(query, values, W1, b1, W2, b2, V, bv)
    res = run_bass_kernel_spmd(nc, in_maps, core_ids=list(range(NCORES)))
    return gather_outputs(res.results)
